# revision 47
# baseline (speedup 1.0000x reference)
"""Trainium2 Bass kernel for nn_AttentionLayer (masked attention pooling).

Reference math (per batch row b):
    pq      = tanh(qe @ Wq.T + bq).reshape(D, H)
    v_b     = pq @ Wr.T                         # collapse H before item
    s       = item_b @ v_b                      # (N,)
    att     = exp(s) * mask   (softmax shift-invariant; no max sub needed)
    denom   = sum(att); if denom < 1e-7*exp(smax): denom += exp(smax)
    out_b   = (att @ item_b) / denom            # (D,)

Fast path (c_max = max unmasked count <= 128, true for the reference
distribution ~Binomial(200, 0.5)): the host packs each row's items into
128 slots (all unmasked + largest-norm masked as fill); the remaining 72
(all masked, only needed for the smax fallback) are shipped transposed
[d, (r, n)] and scored on the PE as block-diagonal matmuls with lhsT=vT.
This halves item DMA, halves DVE score work (one 128-partition tile per
group instead of 128+72) and halves PE pooling (single K=128 accumulate).

v broadcast: instead of the DRAM bounce (8.4 MB of replicated reads),
v is transposed on-chip (vT [d, row]) and broadcast by 128 K=1 PE
matmuls (lhsT=ones[1,128], rhs=vT[d, r-window]) into PSUM; the PSUM
waves are evacuated to a bf16 vd tile by ACT/GPSIMD strided copies.

Slow path (c_max > 128): original unpacked two-tile module (code kept
below) with per-row item permutation - mathematically identical.

Distribution: pure data-parallel over batch across 8 cores (256 rows each).
"""

import sys

if "/opt/trn_rl_repo" not in sys.path:
    sys.path.insert(0, "/opt/trn_rl_repo")

from contextlib import ExitStack

import numpy as np

import concourse.bass as bass
import concourse.bacc as bacc
import concourse.tile as tile
from concourse import masks, mybir

B, N, D, H = 2048, 200, 128, 8
NCORES = 8
BS = B // NCORES          # 256 batch rows per core
P = 128                   # partitions
DH = D * H                # 1024
R = 64                    # rows per group
NG = BS // R              # 4 groups per core
NP = 128                  # packed items per row (fast path)
NE = N - NP               # excluded (masked) items per row = 72
NBK = R // 16             # 4 (16 rows per psum "bank-fill" in extraction)

F32 = mybir.dt.float32
BF16 = mybir.dt.bfloat16
AX = mybir.AxisListType
OP = mybir.AluOpType
ACT = mybir.ActivationFunctionType

_CACHE = {}


def build_module_packed() -> bass.Bass:
    nc = bacc.Bacc("TRN2", target_bir_lowering=False)

    item_t = nc.declare_dram_parameter("item_t", [NP, BS * D], BF16, isOutput=False)
    exclT_in = nc.declare_dram_parameter("exclT", [D, BS * NE], BF16, isOutput=False)
    maskT = nc.declare_dram_parameter("maskT", [NP, BS], BF16, isOutput=False)
    bq = nc.declare_dram_parameter("bq", [1, DH], BF16, isOutput=False)
    wr_rep_in = nc.declare_dram_parameter("Wr_rep", [1, DH], BF16, isOutput=False)
    wqT_in = nc.declare_dram_parameter("WqT", [D, DH], BF16, isOutput=False)
    qeT_in = nc.declare_dram_parameter("qeT", [D, BS], BF16, isOutput=False)
    out = nc.declare_dram_parameter("out", [BS, D], F32, isOutput=True)

    with tile.TileContext(nc) as tc, ExitStack() as ctx:
        const = ctx.enter_context(tc.tile_pool(name="const", bufs=1))
        # psA doubles as: projection psum, v-transpose psum, then the
        # excl-score psum (2-bank rotation) - startup users finish first.
        psA = ctx.enter_context(tc.tile_pool(name="psA", bufs=2, space="PSUM"))
        psP = ctx.enter_context(tc.tile_pool(name="psP", bufs=2, space="PSUM"))
        psC = ctx.enter_context(tc.tile_pool(name="psC", bufs=2, space="PSUM"))
        dram = ctx.enter_context(tc.tile_pool(name="dram", bufs=1, space="DRAM"))
        items = ctx.enter_context(tc.tile_pool(name="items", bufs=3))
        excls = ctx.enter_context(tc.tile_pool(name="excls", bufs=2))
        vden = ctx.enter_context(tc.tile_pool(name="vden", bufs=2))
        tmps = ctx.enter_context(tc.tile_pool(name="tmps", bufs=1))
        setup = ctx.enter_context(tc.tile_pool(name="setup", bufs=1))
        work = ctx.enter_context(tc.tile_pool(name="work", bufs=2))
        exb = ctx.enter_context(tc.tile_pool(name="exb", bufs=2))
        smxp = ctx.enter_context(tc.tile_pool(name="smxp", bufs=4))
        ebp = ctx.enter_context(tc.tile_pool(name="ebp", bufs=4))
        small = ctx.enter_context(tc.tile_pool(name="small", bufs=4))

        # ---- preamble DMAs: projection operands first (they gate the v
        # chain), then mask, then the first item tiles.
        wqT = const.tile([P, DH], BF16)
        nc.sync.dma_start(wqT[:], wqT_in[:])
        qeT_all = const.tile([P, BS], BF16)
        nc.sync.dma_start(qeT_all[:], qeT_in[:])
        bq_sb = const.tile([1, DH], BF16)
        nc.sync.dma_start(bq_sb[:], bq[:])
        wr_rep = const.tile([P, DH], BF16)
        nc.sync.dma_start(wr_rep[:], wr_rep_in[0:1, :].to_broadcast([P, DH]))
        maskT_sb = const.tile([P, BS], BF16)
        nc.sync.dma_start(maskT_sb[:], maskT[:])

        ones1 = const.tile([1, P], BF16)
        nc.vector.memset(ones1[:], 1.0)
        onesK = const.tile([P, 1], BF16)
        nc.vector.memset(onesK[:], 1.0)
        ident = const.tile([P, P], F32)
        masks.make_identity(nc, ident[:])
        identb = const.tile([P, P], BF16)
        nc.vector.tensor_copy(out=identb[:], in_=ident[:])
        # tiny dummy exp to pull the exp_and_others ACT table load (which
        # also contains tanh) into the startup shadow.
        dummy = const.tile([1, 1], F32)
        nc.vector.memset(dummy[:], 0.0)
        nc.scalar.activation(dummy[:], dummy[:], ACT.Exp)

        it_tiles = {}
        ex_tiles = {}

        def issue_item_dma(g):
            it0 = items.tile([NP, R * D], BF16, tag="it0")
            src0 = item_t[:, g * R * D:(g + 1) * R * D]
            hw = R * D // 2
            nc.sync.dma_start(it0[:, 0:hw], src0[:, 0:hw])
            nc.sync.dma_start(it0[:, hw:], src0[:, hw:])
            it_tiles[g] = it0
            ex = excls.tile([P, R * NE], BF16, tag="ex")
            nc.sync.dma_start(ex[:], exclT_in[:, g * R * NE:(g + 1) * R * NE])
            ex_tiles[g] = ex

        issue_item_dma(0)

        # ---- query projection -> v (row-major [row, D]), per 128-row half
        vbs = []
        for half in range(2):
            rows = slice(half * P, (half + 1) * P)
            pqt = setup.tile([P, DH], BF16, tag="pqt")
            for j in range(2):
                js = slice(j * 512, (j + 1) * 512)
                pq_ps = psA.tile([P, 512], F32, tag="pq")
                nc.tensor.matmul(
                    pq_ps[:], qeT_all[:, rows], wqT[:, js], start=True, stop=False)
                nc.tensor.matmul(
                    pq_ps[:], ones1[:], bq_sb[:, js], start=False, stop=True)
                nc.scalar.activation(pqt[:, js], pq_ps[:], ACT.Tanh)
            tmpv = setup.tile([P, DH], BF16, tag="tmpv")
            tmpv3 = tmpv[:].rearrange("p (d h) -> p d h", h=H)
            nc.vector.tensor_tensor(tmpv[:], pqt[:], wr_rep[:], OP.mult)
            v_f32 = setup.tile([P, D], F32, tag="vf")
            nc.vector.tensor_reduce(v_f32[:], tmpv3, axis=AX.X, op=OP.add)
            vb = setup.tile([P, D], BF16, tag=f"vb{half}")
            nc.vector.tensor_copy(out=vb[:], in_=v_f32[:])
            vbs.append(vb)

        # vT [d, row] via PE transposes; evacuate to SBUF bf16 (ACT).
        # 32 zero pad columns allow M=32 excl-matmul lhsT windows.
        vT = const.tile([P, BS + 32], BF16)
        nc.vector.memset(vT[:, BS:BS + 32], 0.0)
        for half in range(2):
            vt_ps = psA.tile([P, P], BF16, tag="pq")
            nc.tensor.transpose(vt_ps[:], vbs[half][:], identb[:])
            nc.scalar.copy(vT[:, half * P:(half + 1) * P], vt_ps[:])

        # v rows -> flat DRAM scratch (row-major); per group a [1, R*D]
        # window lands on partition 0 and K=1 broadcast matmuls stream it.
        vdram = dram.tile([1, BS * D], BF16)
        for half in range(2):
            nc.sync.dma_start(
                vdram[0:1, half * P * D:(half + 1) * P * D], vbs[half][:])

        # per-group 1/denom strips bounce through DRAM (for the [16, NBK]
        # arrangement used by the normalize stage)
        invdram = dram.tile([1, BS], F32)
        smxdram = dram.tile([1, BS], F32)

        # ---- vd broadcast for group g: a 16-partition stride-0 read
        # from DRAM, then 3 SBUF->SBUF partition-doubling DMAs. No HBM
        # re-reads beyond 256 KB/group, no engine time at all.
        def issue_vd(g, vd):
            nc.sync.dma_start(
                vd[0:16, :],
                vdram[0:1, g * R * D:(g + 1) * R * D].to_broadcast(
                    [16, R * D]))
            for k in (16, 32, 64):
                nc.sync.dma_start(vd[k:2 * k, :], vd[0:k, :])

        tails = []
        vd_tiles = {}

        def issue_vd_group(g):
            vd = vden.tile([P, R * D], BF16, tag="vd", name=f"vd{g}")
            issue_vd(g, vd)
            vd_tiles[g] = vd

        issue_vd_group(0)

        atts = {}
        combs = {}

        def emit_pooling(g):
            """dn + 16 M=32 pooling blocks for group g (emitted one group
            late so the PE never stalls on this group's att): each psum
            bank-fill is copied to a chunk tile (ACT) and its diagonal
            strips DMA'd into acc2 right away."""
            att = atts.pop(g)
            it0 = it_tiles.pop(g)
            dn = combs[g][1]
            nc.tensor.matmul(dn, att[:, 0:R], onesK[:], start=True, stop=True)
            acc2 = ebp.tile([16, NBK * D], F32, tag="acc2", name=f"acc2_{g}")
            for f in range(NBK):
                pb = psP.tile([P, 512], F32, tag="pb")
                for jj in range(4):
                    b = 4 * f + jj
                    o = pb[32 * jj:32 * jj + 32, :]
                    nc.tensor.matmul(
                        o, att[:, 4 * b:4 * b + 32],
                        it0[:, 4 * b * D:(4 * b + 4) * D],
                        start=True, stop=True, tile_position=(0, 32 * jj))
                chunk = exb.tile([P, 512], F32, tag="chunk")
                nc.scalar.copy(chunk[:], pb[:])
                for c in range(4):
                    nc.sync.dma_start(
                        acc2[c::4, f * D:(f + 1) * D],
                        chunk[c::32, c * D:(c + 1) * D])
            return acc2

        for g in range(NG):
            for gn in (g, g + 1):
                if gn < NG and gn not in it_tiles:
                    issue_item_dma(gn)
            if g + 1 < NG:
                issue_vd_group(g + 1)
            it0 = it_tiles[g]
            ex = ex_tiles[g]
            vd = vd_tiles.pop(g)

            comb = psC.tile([R, NP + 1], F32, tag="comb")
            s0T = comb[:, 0:NP]
            dn = comb[:, NP:NP + 1]
            combs[g] = (s0T, dn)

            # ---- packed scores on DVE: mult + halving tree + reduce
            tmp = tmps.tile([NP, R * D], BF16, tag="tmp")
            nc.vector.tensor_tensor(tmp[:], it0[:], vd[:], OP.mult)
            t3 = tmp[:].rearrange("p (r d) -> p r d", d=D)
            dd = D
            while dd > 8:
                dd //= 2
                nc.vector.tensor_tensor(
                    t3[:, :, 0:dd], t3[:, :, 0:dd], t3[:, :, dd:2 * dd], OP.add)
            s0 = work.tile([NP, R], F32, tag="s0")
            nc.vector.tensor_reduce(s0[:], t3[:, :, 0:8], axis=AX.X, op=OP.add)
            e0 = work.tile([NP, R], BF16, tag="e0")
            nc.scalar.activation(e0[:], s0[:], ACT.Exp)
            # att padded with 28 zero columns so pooling matmuls can use
            # M=32 windows (writes whole psum partitions - rows past the
            # 4 real ones are dupes/zeros, never read).
            att = work.tile([NP, R + 28], BF16, tag="att")
            nc.vector.memset(att[:, R:R + 28], 0.0)
            nc.vector.tensor_tensor(
                att[:, 0:R], e0[:], maskT_sb[:, g * R:(g + 1) * R], OP.mult)
            atts[g] = att

            # ---- excluded-item scores on PE (emitted before the
            # att-gated ops so the in-order PE stream has ready work):
            # M=32 blocks; row r's own score lands at psum partition
            # 32*jj col window w; full-partition evac (ACT), max-reduce
            # (DVE) -> smx4 rows {0,32,64,96}; DRAM bounce -> smx [R, 1].
            smx4 = work.tile([P, 16], F32, tag="smx4")
            for bk, (w0, nw) in enumerate(((0, 7), (7, 7), (14, 2))):
                exw = psA.tile([P, 512], F32, tag="pq", name=f"ex{g}_{bk}")
                for w_in in range(nw):
                    w = w0 + w_in
                    for jj in range(4):
                        r = 16 * jj + w
                        nc.tensor.matmul(
                            exw[32 * jj:32 * jj + 32,
                                NE * w_in:NE * w_in + NE],
                            vT[:, g * R + r:g * R + r + 32],
                            ex[:, r * NE:(r + 1) * NE],
                            start=True, stop=True, tile_position=(0, 32 * jj))
                exs = exb.tile([P, 7 * NE], F32, tag="exs")
                nc.scalar.copy(exs[:, 0:NE * nw], exw[:, 0:NE * nw])
                nc.vector.tensor_reduce(
                    smx4[:, w0:w0 + nw],
                    exs[:, 0:NE * nw].rearrange("p (w n) -> p w n", n=NE),
                    axis=AX.X, op=OP.max)
            smx = smxp.tile([R, 1], F32, tag="smx", name=f"smx{g}")
            nc.sync.dma_start(
                bass.AP(tensor=smxdram[:].tensor,
                        offset=smxdram[:].offset + g * R,
                        ap=[[16, 4], [1, 16]]),
                smx4[0::32, 0:16])
            nc.sync.dma_start(
                smx[:], smxdram[0:1, g * R:(g + 1) * R])

            # ---- lagged pooling for the previous group, then its tail
            if g >= 1:
                acc2_prev = emit_pooling(g - 1)
                tails[g - 1](acc2_prev)
                tails[g - 1] = None

            nc.tensor.transpose(s0T, s0[:], ident[:])

            # ---- tail: smax (packed + excluded), denom fallback, inv,
            # normalize, store.
            def make_tail(g, s0T, dn, smx):
                def tail(acc2):
                    smax = small.tile([R, 1], F32, tag="sm")
                    nc.vector.tensor_reduce(smax[:], s0T, axis=AX.X, op=OP.max)
                    nc.vector.tensor_tensor(smax[:], smax[:], smx[:], OP.max)
                    es = small.tile([R, 1], F32, tag="es")
                    nc.scalar.activation(es[:], smax[:], ACT.Exp)
                    thr = small.tile([R, 1], F32, tag="th")
                    nc.vector.tensor_scalar(thr[:], es[:], 1e-7, None, OP.mult)
                    dn2 = small.tile([R, 1], F32, tag="dn2")
                    nc.vector.scalar_tensor_tensor(
                        dn2[:], dn, thr[:], es[:], op0=OP.is_lt, op1=OP.mult)
                    nc.vector.tensor_tensor(dn2[:], dn2[:], dn, OP.add)
                    inv = small.tile([R, 1], F32, tag="iv")
                    nc.vector.reciprocal(inv[:], dn2[:])
                    nc.sync.dma_start(invdram[0:1, g * R:(g + 1) * R], inv[:])
                    # inva[p, Bk] = inv[16Bk+p]: four tiny contiguous reads
                    inva = work.tile([16, NBK], F32, tag="inva")
                    for Bk in range(NBK):
                        nc.sync.dma_start(
                            inva[:, Bk:Bk + 1],
                            invdram[0:1, g * R + 16 * Bk:g * R + 16 * Bk + 16])
                    for Bk in range(NBK):
                        nc.vector.tensor_scalar(
                            acc2[:, Bk * D:(Bk + 1) * D],
                            acc2[:, Bk * D:(Bk + 1) * D],
                            inva[:, Bk:Bk + 1], None, OP.mult)
                    dst_o = bass.AP(
                        tensor=out[:].tensor,
                        offset=out[:].offset + g * R * D,
                        ap=[[D, 16], [16 * D, NBK], [1, D]])
                    src_o = acc2[:, :].rearrange("p (b d) -> p b d", d=D)
                    nc.sync.dma_start(dst_o, src_o)
                return tail
            tails.append(make_tail(g, s0T, dn, smx))
        acc2_last = emit_pooling(NG - 1)
        tails[NG - 1](acc2_last)

    nc.compile()
    return nc


def build_module_full() -> bass.Bass:
    """Slow-path module: original unpacked two-tile kernel (any c_max)."""
    nc = bacc.Bacc("TRN2", target_bir_lowering=False)
    N0 = 128
    N1 = N - N0

    item_t = nc.declare_dram_parameter("item_t", [N, BS * D], BF16, isOutput=False)
    maskT = nc.declare_dram_parameter("maskT", [N, BS], BF16, isOutput=False)
    bq = nc.declare_dram_parameter("bq", [1, DH], BF16, isOutput=False)
    wr_rep_in = nc.declare_dram_parameter("Wr_rep", [1, DH], BF16, isOutput=False)
    wqT_in = nc.declare_dram_parameter("WqT", [D, DH], BF16, isOutput=False)
    qeT_in = nc.declare_dram_parameter("qeT", [D, BS], BF16, isOutput=False)
    out = nc.declare_dram_parameter("out", [BS, D], F32, isOutput=True)

    with tile.TileContext(nc) as tc, ExitStack() as ctx:
        const = ctx.enter_context(tc.tile_pool(name="const", bufs=1))
        psA = ctx.enter_context(tc.tile_pool(name="psA", bufs=2, space="PSUM"))
        psP = ctx.enter_context(tc.tile_pool(name="psP", bufs=NBK, space="PSUM"))
        psC = ctx.enter_context(tc.tile_pool(name="psC", bufs=2, space="PSUM"))
        dram = ctx.enter_context(tc.tile_pool(name="dram", bufs=1, space="DRAM"))
        items = ctx.enter_context(tc.tile_pool(name="items", bufs=3))
        vden = ctx.enter_context(tc.tile_pool(name="vden", bufs=2))
        tmps = ctx.enter_context(tc.tile_pool(name="tmps", bufs=1))
        work = ctx.enter_context(tc.tile_pool(name="work", bufs=2))
        small = ctx.enter_context(tc.tile_pool(name="small", bufs=4))

        wqT = const.tile([P, DH], BF16)
        nc.sync.dma_start(wqT[:], wqT_in[:])
        qeT_all = const.tile([P, BS], BF16)
        nc.sync.dma_start(qeT_all[:], qeT_in[:])
        bq_sb = const.tile([1, DH], BF16)
        nc.sync.dma_start(bq_sb[:], bq[:])
        wr_rep = const.tile([P, DH], BF16)
        nc.sync.dma_start(wr_rep[:], wr_rep_in[0:1, :].to_broadcast([P, DH]))
        maskT_sb = const.tile([P, BS], BF16)
        nc.sync.dma_start(maskT_sb[:], maskT[0:N0, :])
        maskT1_sb = const.tile([N1, BS], BF16)
        nc.sync.dma_start(maskT1_sb[:], maskT[N0:N, :])

        ones1 = const.tile([1, P], BF16)
        nc.vector.memset(ones1[:], 1.0)
        onesK = const.tile([P, 1], BF16)
        nc.vector.memset(onesK[:], 1.0)
        ident = const.tile([P, P], F32)
        masks.make_identity(nc, ident[:])

        it_tiles = {}

        def issue_item_dma(g):
            it0 = items.tile([N0, R * D], BF16, tag="it0")
            src0 = item_t[0:N0, g * R * D:(g + 1) * R * D]
            hw = R * D // 2
            nc.sync.dma_start(it0[:, 0:hw], src0[:, 0:hw])
            nc.sync.dma_start(it0[:, hw:], src0[:, hw:])
            it1 = items.tile([N1, R * D], BF16, tag="it1")
            src1 = item_t[N0:N, g * R * D:(g + 1) * R * D]
            nc.sync.dma_start(it1[:, 0:hw], src1[:, 0:hw])
            nc.sync.dma_start(it1[:, hw:], src1[:, hw:])
            it_tiles[g] = (it0, it1)

        issue_item_dma(0)

        vbs = []
        for half in range(2):
            rows = slice(half * P, (half + 1) * P)
            pqt = work.tile([P, DH], BF16, tag="pqt")
            for j in range(2):
                js = slice(j * 512, (j + 1) * 512)
                pq_ps = psA.tile([P, 512], F32, tag="pq")
                nc.tensor.matmul(
                    pq_ps[:], qeT_all[:, rows], wqT[:, js], start=True, stop=False)
                nc.tensor.matmul(
                    pq_ps[:], ones1[:], bq_sb[:, js], start=False, stop=True)
                nc.scalar.activation(pqt[:, js], pq_ps[:], ACT.Tanh)
            tmpv = work.tile([P, DH], BF16, tag="tmpv")
            tmpv3 = tmpv[:].rearrange("p (d h) -> p d h", h=H)
            nc.vector.tensor_tensor(tmpv[:], pqt[:], wr_rep[:], OP.mult)
            v_f32 = work.tile([P, D], F32, tag="vf")
            nc.vector.tensor_reduce(v_f32[:], tmpv3, axis=AX.X, op=OP.add)
            vb = work.tile([P, D], BF16, tag="vb")
            nc.vector.tensor_copy(out=vb[:], in_=v_f32[:])
            vbs.append(vb)

        vdram = dram.tile([1, BS * D], BF16)
        for half in range(2):
            nc.sync.dma_start(
                vdram[0:1, half * P * D:(half + 1) * P * D], vbs[half][:])
        invdram = dram.tile([1, BS], F32)

        tails = []

        def s_phase(g, ti, it, np_, mk, vd):
            tmp = tmps.tile([np_, R * D], BF16, tag=f"tmp{ti}")
            nc.vector.tensor_tensor(tmp[:], it[:], vd[0:np_, :], OP.mult)
            t3 = tmp[:].rearrange("p (r d) -> p r d", d=D)
            dd = D
            while dd > 8:
                dd //= 2
                nc.vector.tensor_tensor(
                    t3[:, :, 0:dd], t3[:, :, 0:dd], t3[:, :, dd:2 * dd], OP.add)
            s = work.tile([np_, R], F32, tag=f"s{ti}")
            nc.vector.tensor_reduce(s[:], t3[:, :, 0:8], axis=AX.X, op=OP.add)
            e = work.tile([np_, R], BF16, tag=f"e{ti}")
            nc.scalar.activation(e[:], s[:], ACT.Exp)
            att = work.tile([np_, R], BF16, tag=f"att{ti}")
            nc.vector.tensor_tensor(
                att[:], e[:], mk[0:np_, g * R:(g + 1) * R], OP.mult)
            return s, att

        for g in range(NG):
            for gn in (g, g + 1, g + 2):
                if gn < NG and gn not in it_tiles:
                    issue_item_dma(gn)
            it0, it1 = it_tiles[g]

            vd = vden.tile([P, R * D], BF16, tag="vd")
            nc.sync.dma_start(
                vd[:],
                vdram[0:1, g * R * D:(g + 1) * R * D].to_broadcast([P, R * D]))

            comb = psC.tile([R, N0 + N1 + 1], F32, tag="comb")
            s0T = comb[:, 0:N0]
            s1T = comb[:, N0:N0 + N1]
            dn = comb[:, N0 + N1:N0 + N1 + 1]
            pbs = [psP.tile([P, 512], F32, tag="pb", name=f"pb{g}_{Bk}")
                   for Bk in range(NBK)]

            s0, att0 = s_phase(g, 0, it0, N0, maskT_sb, vd)
            nc.tensor.transpose(s0T, s0[:], ident[:])
            s1, att1 = s_phase(g, 1, it1, N1, maskT1_sb, vd)
            nc.tensor.transpose(s1T, s1[:], ident[0:N1, 0:N1])
            nc.tensor.matmul(dn, att0[:], onesK[:], start=True, stop=False)
            nc.tensor.matmul(dn, att1[:], onesK[0:N1, :], start=False, stop=True)
            for b in range(4 * NBK):
                o = pbs[b // 4][32 * (b % 4):32 * (b % 4) + 4, :]
                nc.tensor.matmul(
                    o, att0[:, 4 * b:4 * b + 4], it0[:, 4 * b * D:(4 * b + 4) * D],
                    start=True, stop=False, tile_position=(0, 32 * (b % 4)))
                nc.tensor.matmul(
                    o, att1[:, 4 * b:4 * b + 4], it1[:, 4 * b * D:(4 * b + 4) * D],
                    start=False, stop=True, tile_position=(0, 32 * (b % 4)))

            def make_tail(g, s0T, s1T, dn, pbs):
                def tail():
                    smax = small.tile([R, 1], F32, tag="sm")
                    nc.vector.tensor_reduce(smax[:], s0T, axis=AX.X, op=OP.max)
                    sm1 = small.tile([R, 1], F32, tag="sm1")
                    nc.vector.tensor_reduce(sm1[:], s1T, axis=AX.X, op=OP.max)
                    nc.vector.tensor_tensor(smax[:], smax[:], sm1[:], OP.max)
                    es = small.tile([R, 1], F32, tag="es")
                    nc.scalar.activation(es[:], smax[:], ACT.Exp)
                    thr = small.tile([R, 1], F32, tag="th")
                    nc.vector.tensor_scalar(thr[:], es[:], 1e-7, None, OP.mult)
                    dn2 = small.tile([R, 1], F32, tag="dn2")
                    nc.vector.scalar_tensor_tensor(
                        dn2[:], dn, thr[:], es[:], op0=OP.is_lt, op1=OP.mult)
                    nc.vector.tensor_tensor(dn2[:], dn2[:], dn, OP.add)
                    inv = small.tile([R, 1], F32, tag="iv")
                    nc.vector.reciprocal(inv[:], dn2[:])
                    nc.sync.dma_start(invdram[0:1, g * R:(g + 1) * R], inv[:])
                    inva = work.tile([16, NBK], F32, tag="inva")
                    for Bk in range(NBK):
                        nc.sync.dma_start(
                            inva[:, Bk:Bk + 1],
                            invdram[0:1, g * R + 16 * Bk:g * R + 16 * Bk + 16])

                    acc2 = work.tile([16, NBK * D], F32, tag="acc2")
                    ebig = work.tile([P, NBK * 512], F32, tag="ebig")
                    for Bk in range(NBK):
                        nc.scalar.copy(
                            ebig[:, Bk * 512:(Bk + 1) * 512], pbs[Bk][:])
                    FW = NBK * 512
                    for c in range(4):
                        src = bass.AP(
                            tensor=ebig[:].tensor,
                            offset=ebig[:].offset + c * FW + c * D,
                            ap=[[32 * FW, 4], [512, NBK], [1, D]])
                        dst = bass.AP(
                            tensor=acc2[:].tensor,
                            offset=acc2[:].offset + c * NBK * D,
                            ap=[[4 * NBK * D, 4], [D, NBK], [1, D]])
                        nc.sync.dma_start(dst, src)
                    for Bk in range(NBK):
                        nc.vector.tensor_scalar(
                            acc2[:, Bk * D:(Bk + 1) * D],
                            acc2[:, Bk * D:(Bk + 1) * D],
                            inva[:, Bk:Bk + 1], None, OP.mult)
                    dst_o = bass.AP(
                        tensor=out[:].tensor,
                        offset=out[:].offset + g * R * D,
                        ap=[[D, 16], [16 * D, NBK], [1, D]])
                    src_o = bass.AP(
                        tensor=acc2[:].tensor, offset=acc2[:].offset,
                        ap=[[NBK * D, 16], [D, NBK], [1, D]])
                    nc.sync.dma_start(dst_o, src_o)
                return tail
            tails.append(make_tail(g, s0T, s1T, dn, pbs))
            if g >= 1:
                tails[g - 1]()
                tails[g - 1] = None
        tails[NG - 1]()

    nc.compile()
    return nc


def _get_module(packed: bool) -> bass.Bass:
    key = "nc_packed" if packed else "nc_full"
    if key not in _CACHE:
        _CACHE[key] = build_module_packed() if packed else build_module_full()
    return _CACHE[key]


def make_in_maps_packed(item_embedding, query_embedding, mask, Wq, bq, Wr):
    import ml_dtypes

    bf16 = ml_dtypes.bfloat16
    item = np.asarray(item_embedding, dtype=np.float32)
    qe = np.asarray(query_embedding, dtype=np.float32)
    mk = np.asarray(mask).reshape(B, N)
    wq = np.asarray(Wq, dtype=np.float32)
    bqr = np.ascontiguousarray(bq.reshape(1, DH)).astype(bf16)
    wr = np.asarray(Wr, dtype=np.float32)
    wr_rep = np.ascontiguousarray(np.tile(wr.reshape(1, H), (1, D))).astype(bf16)
    wqT = np.ascontiguousarray(wq.T).astype(bf16)

    # pack: per row, all unmasked items first, then masked by descending
    # norm; first NP go to the packed tile, the rest (all masked) to excl.
    norms = np.einsum('bnd,bnd->bn', item, item)
    key = np.where(mk, -np.inf, -norms)
    order = np.argsort(key, axis=1, kind='stable')
    bi = np.arange(B)[:, None]
    keep = order[:, :NP]
    excl = order[:, NP:]
    item_pk = item[bi, keep].astype(bf16)          # (B, NP, D)
    item_ex = item[bi, excl].astype(bf16)          # (B, NE, D)
    mk_pk = mk[bi, keep]                            # (B, NP)

    in_maps = []
    for i in range(NCORES):
        r = slice(i * BS, (i + 1) * BS)
        it = np.ascontiguousarray(
            item_pk[r].transpose(1, 0, 2)).reshape(NP, BS * D)
        ex = np.ascontiguousarray(
            item_ex[r].transpose(2, 0, 1)).reshape(D, BS * NE)
        mt = np.ascontiguousarray(mk_pk[r].T.astype(bf16))
        in_maps.append({
            "item_t": it,
            "exclT": ex,
            "maskT": mt,
            "bq": bqr,
            "Wr_rep": wr_rep,
            "WqT": wqT,
            "qeT": np.ascontiguousarray(qe[r].T.astype(bf16)),
        })
    return in_maps


def make_in_maps_full(item_embedding, query_embedding, mask, Wq, bq, Wr):
    import ml_dtypes

    bf16 = ml_dtypes.bfloat16
    item = np.asarray(item_embedding, dtype=np.float32)
    qe = np.asarray(query_embedding, dtype=np.float32)
    mk = np.asarray(mask).reshape(B, N)
    wq = np.asarray(Wq, dtype=np.float32)
    bqr = np.ascontiguousarray(bq.reshape(1, DH)).astype(bf16)
    wr = np.asarray(Wr, dtype=np.float32)
    wr_rep = np.ascontiguousarray(np.tile(wr.reshape(1, H), (1, D))).astype(bf16)
    wqT = np.ascontiguousarray(wq.T).astype(bf16)
    in_maps = []
    for i in range(NCORES):
        r = slice(i * BS, (i + 1) * BS)
        it = np.ascontiguousarray(
            item[r].astype(bf16).transpose(1, 0, 2)).reshape(N, BS * D)
        mt = np.ascontiguousarray(mk[r].T.astype(bf16))
        in_maps.append({
            "item_t": it,
            "maskT": mt,
            "bq": bqr,
            "Wr_rep": wr_rep,
            "WqT": wqT,
            "qeT": np.ascontiguousarray(qe[r].T.astype(bf16)),
        })
    return in_maps


def kernel(item_embedding, query_embedding, mask, Wq, bq, Wr):
    from concourse.bass_utils import run_bass_kernel_spmd

    mk = np.asarray(mask).reshape(B, N)
    packed = int(mk.sum(axis=1).max()) <= NP
    nc = _get_module(packed)
    if packed:
        in_maps = make_in_maps_packed(
            item_embedding, query_embedding, mask, Wq, bq, Wr)
    else:
        in_maps = make_in_maps_full(
            item_embedding, query_embedding, mask, Wq, bq, Wr)
    last_err = None
    for attempt in range(3):
        try:
            res = run_bass_kernel_spmd(
                nc, in_maps, core_ids=list(range(NCORES)),
                **_CACHE.get("run_kwargs", {})
            )
            break
        except Exception as e:  # transient NRT_EXEC_UNIT_UNRECOVERABLE flakes
            last_err = e
    else:
        raise last_err
    _CACHE["last_results"] = res
    return np.concatenate([res.results[i]["out"] for i in range(NCORES)], axis=0)


# revision 56
# speedup vs baseline: 1.1300x; 1.1300x over previous
"""Trainium2 Bass kernel for nn_AttentionLayer (masked attention pooling).

Reference math (per batch row b):
    pq      = tanh(qe @ Wq.T + bq).reshape(D, H)
    v_b     = pq @ Wr.T                         # collapse H before item
    s       = item_b @ v_b                      # (N,)
    att     = exp(s) * mask   (softmax shift-invariant; no max sub needed)
    denom   = sum(att); if denom < 1e-7*exp(smax): denom += exp(smax)
    out_b   = (att @ item_b) / denom            # (D,)

Fast path (c_max = max unmasked count <= 128, true for the reference
distribution ~Binomial(200, 0.5)): the host packs each row's items into
128 slots (all unmasked + largest-norm masked as fill); the remaining 72
(all masked, only needed for the smax fallback) are shipped transposed
[d, (r, n)] and scored on the PE as block-diagonal matmuls with lhsT=vT.
This halves item DMA, halves DVE score work (one 128-partition tile per
group instead of 128+72) and halves PE pooling (single K=128 accumulate).

v broadcast: instead of the DRAM bounce (8.4 MB of replicated reads),
v is transposed on-chip (vT [d, row]) and broadcast by 128 K=1 PE
matmuls (lhsT=ones[1,128], rhs=vT[d, r-window]) into PSUM; the PSUM
waves are evacuated to a bf16 vd tile by ACT/GPSIMD strided copies.

Slow path (c_max > 128): original unpacked two-tile module (code kept
below) with per-row item permutation - mathematically identical.

Distribution: pure data-parallel over batch across 8 cores (256 rows each).
"""

import sys

if "/opt/trn_rl_repo" not in sys.path:
    sys.path.insert(0, "/opt/trn_rl_repo")

from contextlib import ExitStack

import numpy as np

import concourse.bass as bass
import concourse.bacc as bacc
import concourse.tile as tile
from concourse import masks, mybir

B, N, D, H = 2048, 200, 128, 8
NCORES = 8
BS = B // NCORES          # 256 batch rows per core
P = 128                   # partitions
DH = D * H                # 1024
R = 64                    # rows per group
NG = BS // R              # 4 groups per core
NP = 128                  # packed items per row (fast path)
NE = N - NP               # excluded (masked) items per row = 72
NBK = R // 16             # 4 (16 rows per psum "bank-fill" in extraction)

F32 = mybir.dt.float32
BF16 = mybir.dt.bfloat16
AX = mybir.AxisListType
OP = mybir.AluOpType
ACT = mybir.ActivationFunctionType

_CACHE = {}


def build_module_packed() -> bass.Bass:
    nc = bacc.Bacc("TRN2", target_bir_lowering=False)

    item_t = nc.declare_dram_parameter("item_t", [NP, BS * D], BF16, isOutput=False)
    exclT_in = nc.declare_dram_parameter("exclT", [D, BS * NE], BF16, isOutput=False)
    maskT = nc.declare_dram_parameter("maskT", [NP, BS], BF16, isOutput=False)
    bq = nc.declare_dram_parameter("bq", [1, DH], BF16, isOutput=False)
    wr_rep_in = nc.declare_dram_parameter("Wr_rep", [1, DH], BF16, isOutput=False)
    wqT_in = nc.declare_dram_parameter("WqT", [D, DH], BF16, isOutput=False)
    qeT_in = nc.declare_dram_parameter("qeT", [D, BS], BF16, isOutput=False)
    out = nc.declare_dram_parameter("out", [BS, D], F32, isOutput=True)

    with tile.TileContext(nc) as tc, ExitStack() as ctx:
        const = ctx.enter_context(tc.tile_pool(name="const", bufs=1))
        # psA doubles as: projection psum, v-transpose psum, then the
        # excl-score psum (2-bank rotation) - startup users finish first.
        psA = ctx.enter_context(tc.tile_pool(name="psA", bufs=2, space="PSUM"))
        psP = ctx.enter_context(tc.tile_pool(name="psP", bufs=2, space="PSUM"))
        psC = ctx.enter_context(tc.tile_pool(name="psC", bufs=2, space="PSUM"))
        dram = ctx.enter_context(tc.tile_pool(name="dram", bufs=1, space="DRAM"))
        items = ctx.enter_context(tc.tile_pool(name="items", bufs=3))
        excls = ctx.enter_context(tc.tile_pool(name="excls", bufs=2))
        vden = ctx.enter_context(tc.tile_pool(name="vden", bufs=2))
        tmps = ctx.enter_context(tc.tile_pool(name="tmps", bufs=1))
        setup = ctx.enter_context(tc.tile_pool(name="setup", bufs=1))
        work = ctx.enter_context(tc.tile_pool(name="work", bufs=2))
        smxp = ctx.enter_context(tc.tile_pool(name="smxp", bufs=4))
        ebp = ctx.enter_context(tc.tile_pool(name="ebp", bufs=2))
        small = ctx.enter_context(tc.tile_pool(name="small", bufs=4))

        # ---- preamble DMAs: projection operands first (they gate the v
        # chain), then mask, then the first item tiles.
        wqT = const.tile([P, DH], BF16)
        nc.sync.dma_start(wqT[:], wqT_in[:])
        qeT_all = const.tile([P, BS], BF16)
        nc.sync.dma_start(qeT_all[:], qeT_in[:])
        bq_sb = const.tile([1, DH], BF16)
        nc.sync.dma_start(bq_sb[:], bq[:])
        wr_rep = const.tile([P, DH], BF16)
        nc.sync.dma_start(wr_rep[:], wr_rep_in[0:1, :].to_broadcast([P, DH]))
        maskT_sb = const.tile([P, BS], BF16)
        nc.sync.dma_start(maskT_sb[:], maskT[:])

        ones1 = const.tile([1, P], BF16)
        nc.vector.memset(ones1[:], 1.0)
        onesK = const.tile([P, 1], BF16)
        nc.vector.memset(onesK[:], 1.0)
        ident = const.tile([P, P], F32)
        masks.make_identity(nc, ident[:])
        identb = const.tile([P, P], BF16)
        nc.vector.tensor_copy(out=identb[:], in_=ident[:])
        # tiny dummy exp to pull the exp_and_others ACT table load (which
        # also contains tanh) into the startup shadow.
        dummy = const.tile([1, 1], F32)
        nc.vector.memset(dummy[:], 0.0)
        nc.scalar.activation(dummy[:], dummy[:], ACT.Exp)

        it_tiles = {}
        ex_tiles = {}

        def issue_item_dma(g):
            it0 = items.tile([NP, R * D], BF16, tag="it0")
            src0 = item_t[:, g * R * D:(g + 1) * R * D]
            hw = R * D // 2
            nc.sync.dma_start(it0[:, 0:hw], src0[:, 0:hw])
            nc.sync.dma_start(it0[:, hw:], src0[:, hw:])
            it_tiles[g] = it0
            ex = excls.tile([P, R * NE], BF16, tag="ex")
            nc.sync.dma_start(ex[:], exclT_in[:, g * R * NE:(g + 1) * R * NE])
            ex_tiles[g] = ex

        issue_item_dma(0)

        # ---- query projection -> v (row-major [row, D]), per 128-row half
        vbs = []
        for half in range(2):
            rows = slice(half * P, (half + 1) * P)
            pqt = setup.tile([P, DH], BF16, tag="pqt")
            for j in range(2):
                js = slice(j * 512, (j + 1) * 512)
                pq_ps = psA.tile([P, 512], F32, tag="pq")
                nc.tensor.matmul(
                    pq_ps[:], qeT_all[:, rows], wqT[:, js], start=True, stop=False)
                nc.tensor.matmul(
                    pq_ps[:], ones1[:], bq_sb[:, js], start=False, stop=True)
                nc.scalar.activation(pqt[:, js], pq_ps[:], ACT.Tanh)
            tmpv = setup.tile([P, DH], BF16, tag="tmpv")
            tmpv3 = tmpv[:].rearrange("p (d h) -> p d h", h=H)
            nc.vector.tensor_tensor(tmpv[:], pqt[:], wr_rep[:], OP.mult)
            v_f32 = setup.tile([P, D], F32, tag="vf")
            nc.vector.tensor_reduce(v_f32[:], tmpv3, axis=AX.X, op=OP.add)
            vb = setup.tile([P, D], BF16, tag=f"vb{half}")
            nc.vector.tensor_copy(out=vb[:], in_=v_f32[:])
            vbs.append(vb)

        # vT [d, row] via PE transposes; evacuate to SBUF bf16 (ACT).
        # 32 zero pad columns allow M=32 excl-matmul lhsT windows.
        vT = const.tile([P, BS + 32], BF16)
        nc.vector.memset(vT[:, BS:BS + 32], 0.0)
        for half in range(2):
            vt_ps = psA.tile([P, P], BF16, tag="pq")
            nc.tensor.transpose(vt_ps[:], vbs[half][:], identb[:])
            nc.scalar.copy(vT[:, half * P:(half + 1) * P], vt_ps[:])

        # v rows -> flat DRAM scratch (row-major); per group a [1, R*D]
        # window lands on partition 0 and K=1 broadcast matmuls stream it.
        vdram = dram.tile([1, BS * D], BF16)
        for half in range(2):
            nc.sync.dma_start(
                vdram[0:1, half * P * D:(half + 1) * P * D], vbs[half][:])

        # per-group 1/denom strips bounce through DRAM (for the [16, NBK]
        # arrangement used by the normalize stage)
        invdram = dram.tile([1, BS], F32)

        # ---- vd broadcast for group g: a 16-partition stride-0 read
        # from DRAM, then 3 SBUF->SBUF partition-doubling DMAs. No HBM
        # re-reads beyond 256 KB/group, no engine time at all.
        def issue_vd(g, vd):
            nc.sync.dma_start(
                vd[0:16, :],
                vdram[0:1, g * R * D:(g + 1) * R * D].to_broadcast(
                    [16, R * D]))
            for k in (16, 32, 64):
                nc.sync.dma_start(vd[k:2 * k, :], vd[0:k, :])

        tails = []
        vd_tiles = {}

        def issue_vd_group(g):
            vd = vden.tile([P, R * D], BF16, tag="vd", name=f"vd{g}")
            issue_vd(g, vd)
            vd_tiles[g] = vd

        issue_vd_group(0)

        atts = {}
        combs = {}

        def emit_pooling(g):
            """dn + 16 M=32 pooling blocks for group g (emitted one group
            late so the PE never stalls on this group's att): each psum
            bank-fill is copied to a chunk tile (ACT) and its diagonal
            strips DMA'd into acc2 right away."""
            att = atts.pop(g)
            it0 = it_tiles.pop(g)
            dn = combs[g][1]
            nc.tensor.matmul(dn, att[:, 0:R], onesK[:], start=True, stop=True)
            acc2 = ebp.tile([16, NBK * D], F32, tag="acc2", name=f"acc2_{g}",
                            bufs=4)
            ebig = ebp.tile([P, NBK * 512], F32, tag="ebig", name=f"ebig{g}")
            for f in range(NBK):
                pb = psP.tile([P, 512], F32, tag="pb")
                for jj in range(4):
                    b = 4 * f + jj
                    o = pb[32 * jj:32 * jj + 32, :]
                    nc.tensor.matmul(
                        o, att[:, 4 * b:4 * b + 32],
                        it0[:, 4 * b * D:(4 * b + 4) * D],
                        start=True, stop=True, tile_position=(0, 32 * jj))
                nc.scalar.copy(ebig[:, f * 512:(f + 1) * 512], pb[:])
            FW = NBK * 512
            for c in range(4):
                nc.sync.dma_start(
                    bass.AP(tensor=acc2[:].tensor,
                            offset=acc2[:].offset + c * NBK * D,
                            ap=[[4 * NBK * D, 4], [D, NBK], [1, D]]),
                    bass.AP(tensor=ebig[:].tensor,
                            offset=ebig[:].offset + c * FW + c * D,
                            ap=[[32 * FW, 4], [512, NBK], [1, D]]))
            return acc2

        for g in range(NG):
            for gn in (g, g + 1):
                if gn < NG and gn not in it_tiles:
                    issue_item_dma(gn)
            if g + 1 < NG:
                issue_vd_group(g + 1)
            it0 = it_tiles[g]
            ex = ex_tiles[g]
            vd = vd_tiles.pop(g)

            comb = psC.tile([R, NP + 1], F32, tag="comb")
            s0T = comb[:, 0:NP]
            dn = comb[:, NP:NP + 1]
            combs[g] = (s0T, dn)

            # ---- packed scores on DVE: mult + halving tree + reduce
            tmp = tmps.tile([NP, R * D], BF16, tag="tmp")
            nc.vector.tensor_tensor(tmp[:], it0[:], vd[:], OP.mult)
            t3 = tmp[:].rearrange("p (r d) -> p r d", d=D)
            dd = D
            while dd > 8:
                dd //= 2
                nc.vector.tensor_tensor(
                    t3[:, :, 0:dd], t3[:, :, 0:dd], t3[:, :, dd:2 * dd], OP.add)
            s0 = work.tile([NP, R], F32, tag="s0")
            nc.vector.tensor_reduce(s0[:], t3[:, :, 0:8], axis=AX.X, op=OP.add)
            e0 = work.tile([NP, R], BF16, tag="e0")
            nc.scalar.activation(e0[:], s0[:], ACT.Exp)
            # att padded with 28 zero columns so pooling matmuls can use
            # M=32 windows (writes whole psum partitions - rows past the
            # 4 real ones are dupes/zeros, never read).
            att = work.tile([NP, R + 28], BF16, tag="att")
            nc.vector.memset(att[:, R:R + 28], 0.0)
            nc.vector.tensor_tensor(
                att[:, 0:R], e0[:], maskT_sb[:, g * R:(g + 1) * R], OP.mult)
            atts[g] = att

            # ---- excluded-item scores on PE (emitted before the
            # att-gated ops so the in-order PE stream has ready work):
            # 16 M=4 blocks; block j = 4f+jj lands rows at psum
            # partitions 32jj+c; full-bank evac (ACT), one 3-dim
            # diagonal DMA per fill into sx [R, NE]; DVE row-max.
            sx = work.tile([R, NE], F32, tag="sx")
            exbig = ebp.tile([P, 4 * 4 * NE], F32, tag="exbig",
                             name=f"exbig{g}")
            for f in range(4):
                exw = psA.tile([P, 512], F32, tag="pq", name=f"ex{g}_{f}")
                for jj in range(4):
                    j = 4 * f + jj
                    nc.tensor.matmul(
                        exw[32 * jj:32 * jj + 4, 0:4 * NE],
                        vT[:, g * R + 4 * j:g * R + 4 * j + 4],
                        ex[:, j * 4 * NE:(j + 1) * 4 * NE],
                        start=True, stop=True, tile_position=(0, 32 * jj))
                nc.scalar.copy(
                    exbig[:, f * 4 * NE:(f + 1) * 4 * NE], exw[:, 0:4 * NE])
            XW = 4 * 4 * NE
            for c in range(4):
                nc.sync.dma_start(
                    bass.AP(tensor=sx[:].tensor,
                            offset=sx[:].offset + c * NE,
                            ap=[[4 * NE, 4], [16 * NE, 4], [1, NE]]),
                    bass.AP(tensor=exbig[:].tensor,
                            offset=exbig[:].offset + c * XW + c * NE,
                            ap=[[32 * XW, 4], [4 * NE, 4], [1, NE]]))
            smx = smxp.tile([R, 1], F32, tag="smx", name=f"smx{g}")
            nc.vector.tensor_reduce(smx[:], sx[:], axis=AX.X, op=OP.max)

            # ---- lagged pooling for the previous group, then its tail
            if g >= 1:
                acc2_prev = emit_pooling(g - 1)
                tails[g - 1](acc2_prev)
                tails[g - 1] = None

            nc.tensor.transpose(s0T, s0[:], ident[:])

            # ---- tail: smax (packed + excluded), denom fallback, inv,
            # normalize, store.
            def make_tail(g, s0T, dn, smx):
                def tail(acc2):
                    smax = small.tile([R, 1], F32, tag="sm")
                    nc.vector.tensor_reduce(smax[:], s0T, axis=AX.X, op=OP.max)
                    nc.vector.tensor_tensor(smax[:], smax[:], smx[:], OP.max)
                    es = small.tile([R, 1], F32, tag="es")
                    nc.scalar.activation(es[:], smax[:], ACT.Exp)
                    thr = small.tile([R, 1], F32, tag="th")
                    nc.vector.tensor_scalar(thr[:], es[:], 1e-7, None, OP.mult)
                    dn2 = small.tile([R, 1], F32, tag="dn2")
                    nc.vector.scalar_tensor_tensor(
                        dn2[:], dn, thr[:], es[:], op0=OP.is_lt, op1=OP.mult)
                    nc.vector.tensor_tensor(dn2[:], dn2[:], dn, OP.add)
                    inv = small.tile([R, 1], F32, tag="iv")
                    nc.vector.reciprocal(inv[:], dn2[:])
                    nc.sync.dma_start(invdram[0:1, g * R:(g + 1) * R], inv[:])
                    # inva[p, Bk] = inv[16Bk+p]: one transposing DRAM read
                    inva = work.tile([16, NBK], F32, tag="inva")
                    nc.sync.dma_start(
                        bass.AP(tensor=inva[:].tensor, offset=inva[:].offset,
                                ap=[[NBK, 16], [1, NBK]]),
                        bass.AP(tensor=invdram[:].tensor,
                                offset=invdram[:].offset + g * R,
                                ap=[[1, 16], [16, NBK]]))
                    for Bk in range(NBK):
                        nc.vector.tensor_scalar(
                            acc2[:, Bk * D:(Bk + 1) * D],
                            acc2[:, Bk * D:(Bk + 1) * D],
                            inva[:, Bk:Bk + 1], None, OP.mult)
                    dst_o = bass.AP(
                        tensor=out[:].tensor,
                        offset=out[:].offset + g * R * D,
                        ap=[[D, 16], [16 * D, NBK], [1, D]])
                    src_o = acc2[:, :].rearrange("p (b d) -> p b d", d=D)
                    nc.sync.dma_start(dst_o, src_o)
                return tail
            tails.append(make_tail(g, s0T, dn, smx))
        acc2_last = emit_pooling(NG - 1)
        tails[NG - 1](acc2_last)

    nc.compile()
    return nc


def build_module_full() -> bass.Bass:
    """Slow-path module: original unpacked two-tile kernel (any c_max)."""
    nc = bacc.Bacc("TRN2", target_bir_lowering=False)
    N0 = 128
    N1 = N - N0

    item_t = nc.declare_dram_parameter("item_t", [N, BS * D], BF16, isOutput=False)
    maskT = nc.declare_dram_parameter("maskT", [N, BS], BF16, isOutput=False)
    bq = nc.declare_dram_parameter("bq", [1, DH], BF16, isOutput=False)
    wr_rep_in = nc.declare_dram_parameter("Wr_rep", [1, DH], BF16, isOutput=False)
    wqT_in = nc.declare_dram_parameter("WqT", [D, DH], BF16, isOutput=False)
    qeT_in = nc.declare_dram_parameter("qeT", [D, BS], BF16, isOutput=False)
    out = nc.declare_dram_parameter("out", [BS, D], F32, isOutput=True)

    with tile.TileContext(nc) as tc, ExitStack() as ctx:
        const = ctx.enter_context(tc.tile_pool(name="const", bufs=1))
        psA = ctx.enter_context(tc.tile_pool(name="psA", bufs=2, space="PSUM"))
        psP = ctx.enter_context(tc.tile_pool(name="psP", bufs=NBK, space="PSUM"))
        psC = ctx.enter_context(tc.tile_pool(name="psC", bufs=2, space="PSUM"))
        dram = ctx.enter_context(tc.tile_pool(name="dram", bufs=1, space="DRAM"))
        items = ctx.enter_context(tc.tile_pool(name="items", bufs=3))
        vden = ctx.enter_context(tc.tile_pool(name="vden", bufs=2))
        tmps = ctx.enter_context(tc.tile_pool(name="tmps", bufs=1))
        work = ctx.enter_context(tc.tile_pool(name="work", bufs=2))
        small = ctx.enter_context(tc.tile_pool(name="small", bufs=4))

        wqT = const.tile([P, DH], BF16)
        nc.sync.dma_start(wqT[:], wqT_in[:])
        qeT_all = const.tile([P, BS], BF16)
        nc.sync.dma_start(qeT_all[:], qeT_in[:])
        bq_sb = const.tile([1, DH], BF16)
        nc.sync.dma_start(bq_sb[:], bq[:])
        wr_rep = const.tile([P, DH], BF16)
        nc.sync.dma_start(wr_rep[:], wr_rep_in[0:1, :].to_broadcast([P, DH]))
        maskT_sb = const.tile([P, BS], BF16)
        nc.sync.dma_start(maskT_sb[:], maskT[0:N0, :])
        maskT1_sb = const.tile([N1, BS], BF16)
        nc.sync.dma_start(maskT1_sb[:], maskT[N0:N, :])

        ones1 = const.tile([1, P], BF16)
        nc.vector.memset(ones1[:], 1.0)
        onesK = const.tile([P, 1], BF16)
        nc.vector.memset(onesK[:], 1.0)
        ident = const.tile([P, P], F32)
        masks.make_identity(nc, ident[:])

        it_tiles = {}

        def issue_item_dma(g):
            it0 = items.tile([N0, R * D], BF16, tag="it0")
            src0 = item_t[0:N0, g * R * D:(g + 1) * R * D]
            hw = R * D // 2
            nc.sync.dma_start(it0[:, 0:hw], src0[:, 0:hw])
            nc.sync.dma_start(it0[:, hw:], src0[:, hw:])
            it1 = items.tile([N1, R * D], BF16, tag="it1")
            src1 = item_t[N0:N, g * R * D:(g + 1) * R * D]
            nc.sync.dma_start(it1[:, 0:hw], src1[:, 0:hw])
            nc.sync.dma_start(it1[:, hw:], src1[:, hw:])
            it_tiles[g] = (it0, it1)

        issue_item_dma(0)

        vbs = []
        for half in range(2):
            rows = slice(half * P, (half + 1) * P)
            pqt = work.tile([P, DH], BF16, tag="pqt")
            for j in range(2):
                js = slice(j * 512, (j + 1) * 512)
                pq_ps = psA.tile([P, 512], F32, tag="pq")
                nc.tensor.matmul(
                    pq_ps[:], qeT_all[:, rows], wqT[:, js], start=True, stop=False)
                nc.tensor.matmul(
                    pq_ps[:], ones1[:], bq_sb[:, js], start=False, stop=True)
                nc.scalar.activation(pqt[:, js], pq_ps[:], ACT.Tanh)
            tmpv = work.tile([P, DH], BF16, tag="tmpv")
            tmpv3 = tmpv[:].rearrange("p (d h) -> p d h", h=H)
            nc.vector.tensor_tensor(tmpv[:], pqt[:], wr_rep[:], OP.mult)
            v_f32 = work.tile([P, D], F32, tag="vf")
            nc.vector.tensor_reduce(v_f32[:], tmpv3, axis=AX.X, op=OP.add)
            vb = work.tile([P, D], BF16, tag="vb")
            nc.vector.tensor_copy(out=vb[:], in_=v_f32[:])
            vbs.append(vb)

        vdram = dram.tile([1, BS * D], BF16)
        for half in range(2):
            nc.sync.dma_start(
                vdram[0:1, half * P * D:(half + 1) * P * D], vbs[half][:])
        invdram = dram.tile([1, BS], F32)

        tails = []

        def s_phase(g, ti, it, np_, mk, vd):
            tmp = tmps.tile([np_, R * D], BF16, tag=f"tmp{ti}")
            nc.vector.tensor_tensor(tmp[:], it[:], vd[0:np_, :], OP.mult)
            t3 = tmp[:].rearrange("p (r d) -> p r d", d=D)
            dd = D
            while dd > 8:
                dd //= 2
                nc.vector.tensor_tensor(
                    t3[:, :, 0:dd], t3[:, :, 0:dd], t3[:, :, dd:2 * dd], OP.add)
            s = work.tile([np_, R], F32, tag=f"s{ti}")
            nc.vector.tensor_reduce(s[:], t3[:, :, 0:8], axis=AX.X, op=OP.add)
            e = work.tile([np_, R], BF16, tag=f"e{ti}")
            nc.scalar.activation(e[:], s[:], ACT.Exp)
            att = work.tile([np_, R], BF16, tag=f"att{ti}")
            nc.vector.tensor_tensor(
                att[:], e[:], mk[0:np_, g * R:(g + 1) * R], OP.mult)
            return s, att

        for g in range(NG):
            for gn in (g, g + 1, g + 2):
                if gn < NG and gn not in it_tiles:
                    issue_item_dma(gn)
            it0, it1 = it_tiles[g]

            vd = vden.tile([P, R * D], BF16, tag="vd")
            nc.sync.dma_start(
                vd[:],
                vdram[0:1, g * R * D:(g + 1) * R * D].to_broadcast([P, R * D]))

            comb = psC.tile([R, N0 + N1 + 1], F32, tag="comb")
            s0T = comb[:, 0:N0]
            s1T = comb[:, N0:N0 + N1]
            dn = comb[:, N0 + N1:N0 + N1 + 1]
            pbs = [psP.tile([P, 512], F32, tag="pb", name=f"pb{g}_{Bk}")
                   for Bk in range(NBK)]

            s0, att0 = s_phase(g, 0, it0, N0, maskT_sb, vd)
            nc.tensor.transpose(s0T, s0[:], ident[:])
            s1, att1 = s_phase(g, 1, it1, N1, maskT1_sb, vd)
            nc.tensor.transpose(s1T, s1[:], ident[0:N1, 0:N1])
            nc.tensor.matmul(dn, att0[:], onesK[:], start=True, stop=False)
            nc.tensor.matmul(dn, att1[:], onesK[0:N1, :], start=False, stop=True)
            for b in range(4 * NBK):
                o = pbs[b // 4][32 * (b % 4):32 * (b % 4) + 4, :]
                nc.tensor.matmul(
                    o, att0[:, 4 * b:4 * b + 4], it0[:, 4 * b * D:(4 * b + 4) * D],
                    start=True, stop=False, tile_position=(0, 32 * (b % 4)))
                nc.tensor.matmul(
                    o, att1[:, 4 * b:4 * b + 4], it1[:, 4 * b * D:(4 * b + 4) * D],
                    start=False, stop=True, tile_position=(0, 32 * (b % 4)))

            def make_tail(g, s0T, s1T, dn, pbs):
                def tail():
                    smax = small.tile([R, 1], F32, tag="sm")
                    nc.vector.tensor_reduce(smax[:], s0T, axis=AX.X, op=OP.max)
                    sm1 = small.tile([R, 1], F32, tag="sm1")
                    nc.vector.tensor_reduce(sm1[:], s1T, axis=AX.X, op=OP.max)
                    nc.vector.tensor_tensor(smax[:], smax[:], sm1[:], OP.max)
                    es = small.tile([R, 1], F32, tag="es")
                    nc.scalar.activation(es[:], smax[:], ACT.Exp)
                    thr = small.tile([R, 1], F32, tag="th")
                    nc.vector.tensor_scalar(thr[:], es[:], 1e-7, None, OP.mult)
                    dn2 = small.tile([R, 1], F32, tag="dn2")
                    nc.vector.scalar_tensor_tensor(
                        dn2[:], dn, thr[:], es[:], op0=OP.is_lt, op1=OP.mult)
                    nc.vector.tensor_tensor(dn2[:], dn2[:], dn, OP.add)
                    inv = small.tile([R, 1], F32, tag="iv")
                    nc.vector.reciprocal(inv[:], dn2[:])
                    nc.sync.dma_start(invdram[0:1, g * R:(g + 1) * R], inv[:])
                    inva = work.tile([16, NBK], F32, tag="inva")
                    for Bk in range(NBK):
                        nc.sync.dma_start(
                            inva[:, Bk:Bk + 1],
                            invdram[0:1, g * R + 16 * Bk:g * R + 16 * Bk + 16])

                    acc2 = work.tile([16, NBK * D], F32, tag="acc2")
                    ebig = work.tile([P, NBK * 512], F32, tag="ebig")
                    for Bk in range(NBK):
                        nc.scalar.copy(
                            ebig[:, Bk * 512:(Bk + 1) * 512], pbs[Bk][:])
                    FW = NBK * 512
                    for c in range(4):
                        src = bass.AP(
                            tensor=ebig[:].tensor,
                            offset=ebig[:].offset + c * FW + c * D,
                            ap=[[32 * FW, 4], [512, NBK], [1, D]])
                        dst = bass.AP(
                            tensor=acc2[:].tensor,
                            offset=acc2[:].offset + c * NBK * D,
                            ap=[[4 * NBK * D, 4], [D, NBK], [1, D]])
                        nc.sync.dma_start(dst, src)
                    for Bk in range(NBK):
                        nc.vector.tensor_scalar(
                            acc2[:, Bk * D:(Bk + 1) * D],
                            acc2[:, Bk * D:(Bk + 1) * D],
                            inva[:, Bk:Bk + 1], None, OP.mult)
                    dst_o = bass.AP(
                        tensor=out[:].tensor,
                        offset=out[:].offset + g * R * D,
                        ap=[[D, 16], [16 * D, NBK], [1, D]])
                    src_o = bass.AP(
                        tensor=acc2[:].tensor, offset=acc2[:].offset,
                        ap=[[NBK * D, 16], [D, NBK], [1, D]])
                    nc.sync.dma_start(dst_o, src_o)
                return tail
            tails.append(make_tail(g, s0T, s1T, dn, pbs))
            if g >= 1:
                tails[g - 1]()
                tails[g - 1] = None
        tails[NG - 1]()

    nc.compile()
    return nc


def _get_module(packed: bool) -> bass.Bass:
    key = "nc_packed" if packed else "nc_full"
    if key not in _CACHE:
        _CACHE[key] = build_module_packed() if packed else build_module_full()
    return _CACHE[key]


def make_in_maps_packed(item_embedding, query_embedding, mask, Wq, bq, Wr):
    import ml_dtypes

    bf16 = ml_dtypes.bfloat16
    item = np.asarray(item_embedding, dtype=np.float32)
    qe = np.asarray(query_embedding, dtype=np.float32)
    mk = np.asarray(mask).reshape(B, N)
    wq = np.asarray(Wq, dtype=np.float32)
    bqr = np.ascontiguousarray(bq.reshape(1, DH)).astype(bf16)
    wr = np.asarray(Wr, dtype=np.float32)
    wr_rep = np.ascontiguousarray(np.tile(wr.reshape(1, H), (1, D))).astype(bf16)
    wqT = np.ascontiguousarray(wq.T).astype(bf16)

    # pack: per row, all unmasked items first, then masked by descending
    # norm; first NP go to the packed tile, the rest (all masked) to excl.
    norms = np.einsum('bnd,bnd->bn', item, item)
    key = np.where(mk, -np.inf, -norms)
    order = np.argsort(key, axis=1, kind='stable')
    bi = np.arange(B)[:, None]
    keep = order[:, :NP]
    excl = order[:, NP:]
    item_pk = item[bi, keep].astype(bf16)          # (B, NP, D)
    item_ex = item[bi, excl].astype(bf16)          # (B, NE, D)
    mk_pk = mk[bi, keep]                            # (B, NP)

    in_maps = []
    for i in range(NCORES):
        r = slice(i * BS, (i + 1) * BS)
        it = np.ascontiguousarray(
            item_pk[r].transpose(1, 0, 2)).reshape(NP, BS * D)
        ex = np.ascontiguousarray(
            item_ex[r].transpose(2, 0, 1)).reshape(D, BS * NE)
        mt = np.ascontiguousarray(mk_pk[r].T.astype(bf16))
        in_maps.append({
            "item_t": it,
            "exclT": ex,
            "maskT": mt,
            "bq": bqr,
            "Wr_rep": wr_rep,
            "WqT": wqT,
            "qeT": np.ascontiguousarray(qe[r].T.astype(bf16)),
        })
    return in_maps


def make_in_maps_full(item_embedding, query_embedding, mask, Wq, bq, Wr):
    import ml_dtypes

    bf16 = ml_dtypes.bfloat16
    item = np.asarray(item_embedding, dtype=np.float32)
    qe = np.asarray(query_embedding, dtype=np.float32)
    mk = np.asarray(mask).reshape(B, N)
    wq = np.asarray(Wq, dtype=np.float32)
    bqr = np.ascontiguousarray(bq.reshape(1, DH)).astype(bf16)
    wr = np.asarray(Wr, dtype=np.float32)
    wr_rep = np.ascontiguousarray(np.tile(wr.reshape(1, H), (1, D))).astype(bf16)
    wqT = np.ascontiguousarray(wq.T).astype(bf16)
    in_maps = []
    for i in range(NCORES):
        r = slice(i * BS, (i + 1) * BS)
        it = np.ascontiguousarray(
            item[r].astype(bf16).transpose(1, 0, 2)).reshape(N, BS * D)
        mt = np.ascontiguousarray(mk[r].T.astype(bf16))
        in_maps.append({
            "item_t": it,
            "maskT": mt,
            "bq": bqr,
            "Wr_rep": wr_rep,
            "WqT": wqT,
            "qeT": np.ascontiguousarray(qe[r].T.astype(bf16)),
        })
    return in_maps


def kernel(item_embedding, query_embedding, mask, Wq, bq, Wr):
    from concourse.bass_utils import run_bass_kernel_spmd

    mk = np.asarray(mask).reshape(B, N)
    packed = int(mk.sum(axis=1).max()) <= NP
    nc = _get_module(packed)
    if packed:
        in_maps = make_in_maps_packed(
            item_embedding, query_embedding, mask, Wq, bq, Wr)
    else:
        in_maps = make_in_maps_full(
            item_embedding, query_embedding, mask, Wq, bq, Wr)
    last_err = None
    for attempt in range(3):
        try:
            res = run_bass_kernel_spmd(
                nc, in_maps, core_ids=list(range(NCORES)),
                **_CACHE.get("run_kwargs", {})
            )
            break
        except Exception as e:  # transient NRT_EXEC_UNIT_UNRECOVERABLE flakes
            last_err = e
    else:
        raise last_err
    _CACHE["last_results"] = res
    return np.concatenate([res.results[i]["out"] for i in range(NCORES)], axis=0)
